# revision 16
# baseline (speedup 1.0000x reference)
"""Trainium2 Bass kernel for nn_Convnet_81862076661945 (topk_masking).

Pipeline (per the reference nn.Module):
  - X [3231, 256] f32 is sliced into 8 overlapping time sections [431, 256]
    (stride 400).
  - Section s is convolved (VALID) with W[s] [128, 1, 32, 16] -> potentials
    [128, 400, 241].
  - spikes = potentials >= 15.0; max-pool over (400, 16) windows -> [128, 1, 15]
  - A stacked k-winner reduction over the 8 sections produces a single int32
    channel index (or -1).

Sharding: section-parallel - core s owns section s. The tiny pooled binary
maps [128, 15] are all-gathered and every core redundantly computes the final
winner on-device.

Conv mapping (per core): fp8(e4m3) DoubleRow matmuls. Contraction 512 taps =
2 h-groups x (2 i-groups x 128 rows (dfc x dt)) where freq tap
df = 4*(2h+i) + dfc. Per pair of output times: 2 PSUM-accumulated DoubleRow
matmuls (each contracting 256) with 512 output columns (2 x 256, cols f>=241
are garbage and never read). The im2col rhs tile is a single strided DMA from
one fp8 copy of the section: partition (dfc, dt) holds a contiguous 2080-byte
run starting at x[t0 + dt, dfc].

Pooling: window max over (t, 16-freq) windows. Per pair, either the vector
engine reduces straight from PSUM, or the scalar engine copies PSUM -> SBUF
bf16 and the vector engine reduces at 4x. Partial maxes accumulate in a
q-major [128, 15*400] bf16 buffer; one 4x reduce collapses it, then one
threshold yields the binary spike map.

Final winner: spike maps all-gathered via collective; per-channel stats
computed with ~30 small vector ops, cross-partition maxima via 3 PE
transposes. total' = n*(val+8); feat from a packed (256*max - channel) trick.
"""

import sys

if "/opt/trn_rl_repo" not in sys.path:
    sys.path.insert(0, "/opt/trn_rl_repo")

import numpy as np
import ml_dtypes

import concourse.bass as bass
import concourse.bacc as bacc
import concourse.mybir as mybir
import concourse.tile as tile
from concourse.bass_utils import run_bass_kernel_spmd
import bass_rust

# problem constants (hardcoded per harness contract)
N_SECTIONS, N_CHANNELS = 8, 128
KT, KF = 32, 16
LPOST = 400                       # output times per section
LPRE = KT + LPOST - 1             # 431 input rows per section
SECTION_DISTANCE = 400
N_TIMESTEPS, FREQ = 3231, 256
THRESHOLD = 15.0
FOUT = FREQ - KF + 1              # 241 output freqs
FP = FOUT // KF                   # 15 pooled freqs
NDFC = 4                          # freq shifts baked into partitions
T_BATCH = 8                       # output times per im2col DMA
N_BATCH = LPOST // T_BATCH        # 50
T_PAIR = 2                        # output times per PSUM bank
PAIRS_PER_BATCH = T_BATCH // T_PAIR
N_PAIRS = LPOST // T_PAIR         # 200
XCOLS = T_BATCH * FREQ + 32       # im2col tile cols (pad for group shifts)

FP8 = mybir.dt.float8e4
BF16 = mybir.dt.bfloat16
F32 = mybir.dt.float32
I32 = mybir.dt.int32
OP = mybir.AluOpType
DR = mybir.MatmulPerfMode.DoubleRow
AF = mybir.ActivationFunctionType


def _sub_ap(t, extra_offset, free_dims):
    """View of an SBUF/PSUM tile with custom (possibly overlapping) free dims."""
    base = t[:]
    return bass_rust.AP(
        base.tensor,
        base.offset + extra_offset,
        [list(base.ap[0])] + [list(d) for d in free_dims],
    )


def _ap(handle, offset, dims):
    """Arbitrary strided access pattern on a DRAM tensor handle."""
    return bass_rust.AP(handle, offset, [list(d) for d in dims])


def build_nc():
    nc = bacc.Bacc(num_devices=N_SECTIONS)

    xs8 = nc.dram_tensor("xs8", [LPRE + 1, FREQ], FP8, kind="ExternalInput")
    wdr = nc.dram_tensor("wdr", [128, 512], FP8, kind="ExternalInput")
    out = nc.dram_tensor("out", [1, 1], I32, kind="ExternalOutput")
    spk_dbg = nc.dram_tensor("spk_dbg", [N_CHANNELS, FP], F32, kind="ExternalOutput")
    cc_in = nc.dram_tensor("cc_in", [N_CHANNELS, FP], F32)
    cc_out = nc.dram_tensor(
        "cc_out", [N_SECTIONS, N_CHANNELS, FP], F32, addr_space="Shared"
    )

    with tile.TileContext(nc) as tc:
        with (
            tc.tile_pool(name="wp", bufs=1) as wp,
            tc.tile_pool(name="xp", bufs=6) as xp,
            tc.tile_pool(name="cp", bufs=4) as cp,
            tc.tile_pool(name="pp", bufs=3, space="PSUM") as pp,
            tc.tile_pool(name="pf", bufs=1, space="PSUM") as pf,
            tc.tile_pool(name="mp", bufs=1) as mpool,
        ):
            # ---- weights: SBUF [p=(dfc,dt)=128, (h, i, c)] fp8 ----
            wtile = wp.tile([128, 512], FP8)
            nc.sync.dma_start(out=wtile[:], in_=wdr[:])

            # ---- iota helpers (overlap with conv) ----
            # iomat[p, j] = p - j ; cidx[p] = p ; idn = (iomat == 0)
            iomat = mpool.tile([128, 128], F32)
            nc.gpsimd.iota(
                iomat[:], [[-1, 128]], base=0, channel_multiplier=1,
                allow_small_or_imprecise_dtypes=True,
            )
            idn = mpool.tile([128, 128], F32)
            nc.vector.tensor_single_scalar(idn[:], iomat[:], 0.0, OP.is_equal)

            # ---- pooling accumulators ----
            # Work unit is a QUAD = 2 pairs = 4 output times = one 2-bank PSUM
            # tile. Quad evacuation cycles through POOL_PATTERN:
            #   V: vector windowed-max reduce straight from PSUM -> macc_v slot
            #   S: scalar copies PSUM -> SBUF bf16, vector max-accumulates
            POOL_PATTERN = ("V", "S", "S")
            N_QUADS = N_PAIRS // 2
            T_QUAD = 2 * T_PAIR
            NV = (N_QUADS + 2) // 3
            macc_v = mpool.tile([128, FP * NV], BF16)
            acc_sb = mpool.tile([128, T_QUAD * FREQ], BF16)
            nc.vector.memset(acc_sb[:], 0.0)

            xs8_h = xs8[:].tensor
            nv_seen = 0

            for b in range(N_BATCH):
                t0 = b * T_BATCH
                xr = xp.tile([128, XCOLS], FP8)
                # partition (dfc, dt) holds xs8[t0 + dt, dfc : dfc + XCOLS]
                src = _ap(
                    xs8_h,
                    t0 * FREQ,
                    [
                        (1, NDFC),        # dfc  (partition, outer)
                        (FREQ, KT),       # dt   (partition, inner)
                        (1, XCOLS),       # contiguous run (free)
                    ],
                )
                # spread im2col DMAs over SWDGE (engines 64-79) and the two
                # HWDGE queues (engines 64-67) ~4:2:1 to engage all engines
                r = b % 7
                dma_eng = nc.gpsimd if r in (0, 2, 4, 6) else (
                    nc.sync if r in (1, 5) else nc.scalar
                )
                dma_eng.dma_start(out=xr[:], in_=src)

                for pq in range(PAIRS_PER_BATCH // 2):
                    quad = b * (PAIRS_PER_BATCH // 2) + pq
                    ps = pp.tile([128, T_QUAD * FREQ], F32)
                    for half in range(2):
                        for h in range(2):
                            # rhs free dims: (i: shift 4, 2) x (cols: 512)
                            rhs = _sub_ap(
                                xr,
                                (2 * pq + half) * (T_PAIR * FREQ) + 8 * h,
                                [(4, 2), (1, T_PAIR * FREQ)],
                            )
                            lhsT = _sub_ap(wtile, 256 * h, [(128, 2), (1, 128)])
                            out_h = _sub_ap(
                                ps, half * (T_PAIR * FREQ), [(1, T_PAIR * FREQ)]
                            )
                            nc.tensor.matmul(
                                out_h, lhsT, rhs,
                                start=(h == 0), stop=(h == 1), perf_mode=DR,
                            )
                    kind = POOL_PATTERN[quad % len(POOL_PATTERN)]
                    if kind == "V":
                        # vector windowed max straight from PSUM -> macc_v slot
                        src_r = _sub_ap(
                            ps, 0, [(KF, FP), (FREQ, T_QUAD), (1, KF)]
                        )
                        dst = _sub_ap(macc_v, nv_seen * FP, [(1, FP)])
                        nc.vector.tensor_reduce(
                            dst, src_r, axis=mybir.AxisListType.XY, op=OP.max
                        )
                        nv_seen += 1
                    else:
                        # scalar copies PSUM -> SBUF bf16; vector max-accumulates
                        sb = cp.tile([128, T_QUAD * FREQ], BF16)
                        nc.scalar.activation(sb[:], ps[:], AF.Copy)
                        nc.vector.tensor_tensor(
                            acc_sb[:], acc_sb[:], sb[:], OP.max
                        )

            # ---- final max merge, then threshold ----
            mpt = mpool.tile([128, FP], BF16)
            nc.vector.tensor_reduce(
                mpt[:], _sub_ap(macc_v, 0, [(1, FP), (FP, NV)]),
                axis=mybir.AxisListType.X, op=OP.max,
            )
            m2 = mpool.tile([128, FP], BF16)
            nc.vector.tensor_reduce(
                m2[:], _sub_ap(acc_sb, 0, [(KF, FP), (FREQ, T_QUAD), (1, KF)]),
                axis=mybir.AxisListType.XY, op=OP.max,
            )
            nc.vector.tensor_tensor(mpt[:], mpt[:], m2[:], OP.max)
            spk_loc = mpool.tile([128, FP], F32)
            nc.vector.tensor_single_scalar(spk_loc[:], mpt[:], THRESHOLD, OP.is_ge)
            nc.sync.dma_start(out=cc_in[:], in_=spk_loc[:])
            nc.sync.dma_start(out=spk_dbg[:], in_=spk_loc[:])

            # ---- all-gather binary spike maps across the 8 cores ----
            nc.gpsimd.collective_compute(
                "AllGather",
                OP.bypass,
                replica_groups=[list(range(N_SECTIONS))],
                ins=[cc_in[:]],
                outs=[cc_out[:]],
            )

            # ---- gather to SBUF: gt[c, (s, q)] (60B runs) ----
            gt = mpool.tile([128, N_SECTIONS * FP], F32)
            gsrc = _ap(
                cc_out[:].tensor,
                0,
                [
                    (FP, N_CHANNELS),             # c (partition)
                    (N_CHANNELS * FP, N_SECTIONS),  # s (free)
                    (1, FP),                      # q (free, contiguous)
                ],
            )
            nc.sync.dma_start(out=gt[:], in_=gsrc)

            # ---- per-(c,q) stats ----
            # n = sum_s spk ; e = min(8-n, 7) ; val = sum_s spk_s * (e == s)
            spk_qs = _sub_ap(gt, 0, [(1, FP), (FP, N_SECTIONS)])
            n_t = mpool.tile([128, FP], F32)
            nc.vector.tensor_reduce(
                n_t[:], spk_qs, axis=mybir.AxisListType.X, op=OP.add
            )
            e_t = mpool.tile([128, FP], F32)
            nc.vector.tensor_scalar(
                e_t[:], n_t[:], float(N_SECTIONS), -1.0, OP.subtract, OP.mult
            )
            nc.vector.tensor_scalar_min(e_t[:], e_t[:], float(N_SECTIONS - 1))

            val = mpool.tile([128, FP], F32)
            nc.vector.memset(val[:], 0.0)
            tmp = mpool.tile([128, FP], F32)
            for s in range(N_SECTIONS):
                nc.vector.scalar_tensor_tensor(
                    tmp[:], e_t[:], float(s), gt[:, s * FP : (s + 1) * FP],
                    OP.is_equal, OP.mult,
                )
                nc.vector.tensor_tensor(val[:], val[:], tmp[:], OP.add)

            # q_t = val * min(n, 1)  (for the global "any winner" test)
            nmin = mpool.tile([128, FP], F32)
            nc.vector.tensor_scalar_min(nmin[:], n_t[:], 1.0)
            q_t = mpool.tile([128, FP], F32)
            nc.vector.tensor_tensor(q_t[:], val[:], nmin[:], OP.mult)
            # tot = n * (val + 8)
            tot = mpool.tile([128, FP], F32)
            nc.vector.tensor_scalar_add(tot[:], val[:], float(N_SECTIONS))
            nc.vector.tensor_tensor(tot[:], tot[:], n_t[:], OP.mult)

            # per-channel maxima [128, 3]: (mq_col, gmax_col, packed_col)
            cols = mpool.tile([128, 3], F32)
            nc.vector.tensor_reduce(
                cols[:, 0:1], q_t[:], axis=mybir.AxisListType.X, op=OP.max
            )
            nc.vector.tensor_reduce(
                cols[:, 1:2], tot[:], axis=mybir.AxisListType.X, op=OP.max
            )
            # packed = 256 * rmax - c   (exact in f32; rmax integer <= 72)
            nc.vector.scalar_tensor_tensor(
                cols[:, 2:3], cols[:, 1:2], 256.0, iomat[:, 0:1],
                OP.mult, OP.subtract,
            )

            # transpose the 3 columns to rows via PE, then reduce across c
            scl = mpool.tile([1, 3], F32)
            for k in range(3):
                pst = pf.tile([1, 128], F32, tag="pt")
                nc.tensor.matmul(
                    pst[:], cols[:, k : k + 1], idn[:], start=True, stop=True
                )
                nc.vector.tensor_reduce(
                    scl[:, k : k + 1], pst[:], axis=mybir.AxisListType.X, op=OP.max
                )

            # feat = 256*gmax - pmax ; g = (mq > 0) ; ans = feat*g + g - 1
            feat = mpool.tile([1, 1], F32)
            nc.vector.scalar_tensor_tensor(
                feat[:], scl[:, 1:2], 256.0, scl[:, 2:3], OP.mult, OP.subtract
            )
            g_t = mpool.tile([1, 1], F32)
            nc.vector.tensor_single_scalar(g_t[:], scl[:, 0:1], 0.0, OP.is_gt)
            ansf = mpool.tile([1, 1], F32)
            nc.vector.tensor_tensor(ansf[:], feat[:], g_t[:], OP.mult)
            nc.vector.tensor_tensor(ansf[:], ansf[:], g_t[:], OP.add)
            nc.vector.tensor_scalar_sub(ansf[:], ansf[:], 1.0)
            ansi = mpool.tile([1, 1], I32)
            nc.vector.tensor_copy(ansi[:], ansf[:])
            nc.sync.dma_start(out=out[:], in_=ansi[:])

    nc.compile()
    return nc


def prep_inputs(X, W):
    """Host-side sharding + layout packing. Returns in_maps for 8 cores."""
    X = np.asarray(X, dtype=np.float32)
    W = np.asarray(W, dtype=np.float32)
    in_maps = []
    for s in range(N_SECTIONS):
        xs = np.zeros((LPRE + 1, FREQ), dtype=np.float32)
        xs[:LPRE] = X[s * SECTION_DISTANCE : s * SECTION_DISTANCE + LPRE]
        # wdr[dfc*32+dt, h*256 + i*128 + c] = W[s, c, 0, dt, 4*(2h+i)+dfc]
        w = W[s, :, 0]                      # [c, dt, df]
        w = w.transpose(2, 1, 0)            # [df, dt, c]
        w = w.reshape(2, 2, NDFC, KT, N_CHANNELS)   # [h, i, dfc, dt, c]
        w = w.transpose(2, 3, 0, 1, 4)      # [dfc, dt, h, i, c]
        wdr = np.ascontiguousarray(w).reshape(128, 512)
        in_maps.append(
            {
                "xs8": xs.astype(ml_dtypes.float8_e4m3),
                "wdr": wdr.astype(ml_dtypes.float8_e4m3),
            }
        )
    return in_maps


_NC_CACHE = {}


def run(X, W, trace=False, **kwargs):
    if "nc" not in _NC_CACHE:
        _NC_CACHE["nc"] = build_nc()
    nc = _NC_CACHE["nc"]
    in_maps = prep_inputs(X, W)
    res = run_bass_kernel_spmd(
        nc, in_maps, core_ids=list(range(N_SECTIONS)), trace=trace, **kwargs
    )
    return np.int32(res.results[0]["out"][0, 0]), res


def kernel(X, W):
    ans, _ = run(X, W)
    return ans


if __name__ == "__main__":
    X = np.random.rand(N_TIMESTEPS, FREQ).astype(np.float32) * 0.073
    W = (0.8 + 0.05 * np.random.randn(N_SECTIONS, N_CHANNELS, 1, KT, KF)).astype(
        np.float32
    )
    print(kernel(X, W))
